# revision 1
# baseline (speedup 1.0000x reference)
"""CFConv (SchNet-style continuous-filter conv) kernel for 8 TRN2 NeuronCores.

Math: the reference computes
    e_k  = exp(-10*(d[b,i,j] - 0.1*k)^2)            k = 0..299
    h    = ssp(e_k @ W1 + b1)                        [B,N,N,64]
    w_l  = ssp(h @ W2 + b2)                          [B,N,N,64]
    out  = sum_j x[b,i,:] * w_l[b,i,j,:]  = x[b,i,:] * sum_j g(d[b,i,j])
where g: scalar -> R^64 is a smooth analytic function of the distance alone
(ssp = softplus - log 2).

g is analytic on d in [0,1), so a degree-14 polynomial approximates it to
~1e-6 (Chebyshev-equivalent accuracy; final rel err ~4e-7).  The device
evaluates a polynomial DAG whose tiles span degrees 0..14 (all Chebyshev-
like, values in [-1,1]):
    u   = 2d - 1                                    (ACT Copy, affine)
    t2  = u^2; t4 = (2 t2 - 1)^2; t8 = (2 t4 - 1)^2 (ACT Square)
    t3 = u*t2, t5 = u*t4, ..., t14 = t6*t8          (DVE scalar_tensor_tensor)
Every op carries the free-dim j-reduction fused via accum_out, yielding
P[i, b, n] = sum_j tile_n directly in SBUF with no separate reduction pass
(PE fp32 matmul is 4x slow, so PE only does the tiny mixing
S = P^T-slices @ A after one [128,64] transpose).  Finally out = x * S.

Host-side, inputs are pre-transposed to the on-chip layout ([i, b, ...]) so
every DMA is a contiguous per-partition stream, and the output is
transposed back after the run.

Sharding: data-parallel over the batch dim B=16 -> 2 batches per core.
"""

import numpy as np

import concourse.bacc as bacc
import concourse.bass as bass
import concourse.masks as masks
import concourse.mybir as mybir
from concourse.bass_utils import run_bass_kernel_spmd
from concourse.tile import TileContext

F32 = mybir.dt.float32
ALU = mybir.AluOpType
AFT = mybir.ActivationFunctionType

N_CORES = 8
B, N, F = 16, 128, 64
B_LOC = B // N_CORES          # batches per core
N_RBF = 300
GAMMA = 10.0
LOG2 = float(np.log(2.0))

M_DEG = 14                    # polynomial degree of the fit
N_BASIS = M_DEG + 1           # constant + degrees 1..M
P_PAD = 32                    # per-batch P block stride (PE partition align)


# ----------------------------------------------------------------------------
# Host-side: replicate the device polynomial DAG and LS-fit g in it
# ----------------------------------------------------------------------------

def _dag_tiles(d, M):
    """degree -> values of the device tile, float64."""
    u = 2.0 * d - 1.0
    tiles = {1: u}
    p = 1
    while 2 * p <= M:
        src = tiles[p]
        tiles[2 * p] = (u * u) if p == 1 else (2.0 * src - 1.0) ** 2
        p *= 2
    for n in range(3, M + 1):
        if n in tiles:
            continue
        hp = 1 << (n.bit_length() - 1)   # largest power of two <= n
        if hp == n:
            continue
        tiles[n] = tiles[hp] * tiles[n - hp]
    return tiles


def _coef_table(W1, b1, W2, b2):
    """A[n, f] so that g_f(d) ~= sum_n A[n, f] * tile_n(d) (float32)."""
    Q = 8192
    dq = np.linspace(0.0, 1.0, Q)

    centers = 0.1 * np.arange(N_RBF)
    e = np.exp(-GAMMA * (dq[:, None] - centers) ** 2)            # [Q, 300]

    def ssp(v):
        return np.logaddexp(0.0, v) - LOG2

    h = ssp(e @ W1.astype(np.float64) + b1.astype(np.float64))
    g = ssp(h @ W2.astype(np.float64) + b2.astype(np.float64))   # [Q, 64]

    tiles = _dag_tiles(dq, M_DEG)
    Bmat = np.stack([np.ones_like(dq)] +
                    [tiles[n] for n in range(1, M_DEG + 1)], 1)  # [Q, N_BASIS]
    A, *_ = np.linalg.lstsq(Bmat, g, rcond=None)
    return np.ascontiguousarray(A, np.float32)


# ----------------------------------------------------------------------------
# Device kernel (per core), all I/O in on-chip layout:
#   d [N, B_LOC, N] (= [i, b, j]), x/y [N, B_LOC, F] (= [i, b, f])
# ----------------------------------------------------------------------------

_NC_CACHE = None


def _build_nc():
    nc = bacc.Bacc()

    d_in = nc.declare_dram_parameter("d", [N, B_LOC, N], F32, isOutput=False)
    x_in = nc.declare_dram_parameter("x", [N, B_LOC, F], F32, isOutput=False)
    a_in = nc.declare_dram_parameter("coef", [2 * P_PAD, F], F32,
                                     isOutput=False)
    y_out = nc.declare_dram_parameter("y", [N, B_LOC, F], F32, isOutput=True)

    with TileContext(nc) as tc:
        with (
            tc.sbuf_pool(name="sb", bufs=1) as sb,
            tc.psum_pool(name="ps", bufs=1) as ps,
        ):
            neg1_sb = sb.tile([N, 1], F32)            # bias for ACT Square
            nc.gpsimd.memset(neg1_sb[:, :], -1.0)

            # ---- loads; d first (it gates all compute) --------------------
            d_sb = sb.tile([N, B_LOC, N], F32)        # [i, (b, j)]
            # per-batch DMAs (full 128 partitions each for full port BW)
            for b in range(B_LOC):
                nc.sync.dma_start(out=d_sb[:, b, :], in_=d_in[:, b, :])
            a_sb = sb.tile([2 * P_PAD, F], F32)       # A rows at 0 and P_PAD
            nc.sync.dma_start(out=a_sb[:, :], in_=a_in[:, :])
            x_sb = sb.tile([N, B_LOC, F], F32)        # [i, (b, f)]
            nc.sync.dma_start(out=x_sb[:, :, :], in_=x_in[:, :, :])

            # identity for the P transpose, built on the idle GPSIMD
            id_sb = sb.tile([N, N], F32)
            masks.make_identity(nc, id_sb[:, :])

            # P[i, b, n] = sum_j tile_n[i, b, j]; per-batch block padded to
            # P_PAD columns so transposed rows land at partitions 0 / P_PAD
            P_sb = sb.tile([N, B_LOC, P_PAD], F32)
            nc.gpsimd.memset(P_sb[:, :, 0:1], float(N))   # constant basis

            # ---- polynomial DAG with fused j-reduction --------------------
            # level-interleaved ACT chain (u -> t2 -> t4 -> t8, b inner) so
            # both batches' products unblock as early as possible
            t = {}
            for n in (1, 2, 4, 8):
                if n <= M_DEG:
                    t[n] = sb.tile([N, B_LOC, N], F32, name=f"t{n}")

            for b in range(B_LOC):
                nc.scalar.activation(t[1][:, b, :], d_sb[:, b, :], AFT.Copy,
                                     bias=-1.0, scale=2.0,
                                     accum_out=P_sb[:, b, 1:2])
            for b in range(B_LOC):
                nc.scalar.activation(t[2][:, b, :], t[1][:, b, :], AFT.Square,
                                     bias=0.0, scale=1.0,
                                     accum_out=P_sb[:, b, 2:3])
            for b in range(B_LOC):
                nc.scalar.activation(t[4][:, b, :], t[2][:, b, :], AFT.Square,
                                     bias=neg1_sb[:, 0:1], scale=2.0,
                                     accum_out=P_sb[:, b, 4:5])
            for b in range(B_LOC):
                nc.scalar.activation(t[8][:, b, :], t[4][:, b, :], AFT.Square,
                                     bias=neg1_sb[:, 0:1], scale=2.0,
                                     accum_out=P_sb[:, b, 8:9])

            for n in range(3, M_DEG + 1):
                if n in t:
                    continue
                hp = 1 << (n.bit_length() - 1)
                if hp == n:
                    continue
                tn = sb.tile([N, B_LOC, N], F32, name=f"t{n}")
                for b in range(B_LOC):
                    # tn = (t_hp * 1.0) * t_{n-hp}, accum_out = sum_j tn
                    nc.vector.scalar_tensor_tensor(
                        tn[:, b, :], t[hp][:, b, :], 1.0, t[n - hp][:, b, :],
                        ALU.mult, ALU.mult,
                        accum_out=P_sb[:, b, n:n + 1])
                t[n] = tn

            # ---- S = P^T-slices @ A per batch (tiny PE work) --------------
            # one transpose for BOTH batches: P [128, (b n)] -> [(b n), 128]
            pt_ps = ps.tile([B_LOC * P_PAD, N], F32, space="PSUM")
            nc.tensor.transpose(pt_ps[:, :],
                                P_sb.rearrange("i b n -> i (b n)"),
                                id_sb[:, :])
            pt_sb = sb.tile([B_LOC * P_PAD, N], F32)
            for b in range(B_LOC):
                nc.vector.tensor_copy(
                    pt_sb[b * P_PAD:b * P_PAD + N_BASIS, :],
                    pt_ps[b * P_PAD:b * P_PAD + N_BASIS, :])

            s_ps = [ps.tile([N, F], F32, space="PSUM", name=f"s_ps{b}")
                    for b in range(B_LOC)]
            o_sb = sb.tile([N, B_LOC, F], F32)
            for b in range(B_LOC):
                nc.tensor.matmul(s_ps[b][:, :],
                                 pt_sb[b * P_PAD:b * P_PAD + N_BASIS, :],
                                 a_sb[b * P_PAD:b * P_PAD + N_BASIS, :])
                # out = x * S
                nc.vector.tensor_tensor(o_sb[:, b, :], s_ps[b][:, :],
                                        x_sb[:, b, :], ALU.mult)
                nc.sync.dma_start(out=y_out[:, b, :], in_=o_sb[:, b, :])

    nc.compile()
    return nc


# ----------------------------------------------------------------------------
# Public entry point
# ----------------------------------------------------------------------------

def _run(x, distances, W1, b1, W2, b2, trace=False, **trace_kwargs):
    global _NC_CACHE
    x = np.asarray(x, np.float32)
    distances = np.asarray(distances, np.float32)

    A = _coef_table(W1, b1, W2, b2)                  # [N_BASIS, F]
    a_pad = np.zeros((2 * P_PAD, F), np.float32)
    a_pad[0:N_BASIS] = A
    a_pad[P_PAD:P_PAD + N_BASIS] = A

    if _NC_CACHE is None:
        _NC_CACHE = _build_nc()
    nc = _NC_CACHE

    in_maps = []
    for c in range(N_CORES):
        sl = slice(c * B_LOC, (c + 1) * B_LOC)
        in_maps.append({
            # pre-transpose to the on-chip [i, b, ...] layout so the DMAs
            # stream contiguously into the partitions
            "d": np.ascontiguousarray(
                distances[sl].transpose(1, 0, 2)),   # [N, B_LOC, N]
            "x": np.ascontiguousarray(x[sl].transpose(1, 0, 2)),
            "coef": a_pad,
        })

    res = run_bass_kernel_spmd(nc, in_maps, list(range(N_CORES)),
                               trace=trace, **trace_kwargs)
    y = np.concatenate(
        [res.results[c]["y"].transpose(1, 0, 2) for c in range(N_CORES)],
        axis=0)
    return np.ascontiguousarray(y), res


def kernel(x, distances, W1, b1, W2, b2):
    y, _ = _run(x, distances, W1, b1, W2, b2)
    return y



# revision 4
# speedup vs baseline: 1.2528x; 1.2528x over previous
"""CFConv (SchNet-style continuous-filter conv) kernel for 8 TRN2 NeuronCores.

Math: the reference computes
    e_k  = exp(-10*(d[b,i,j] - 0.1*k)^2)            k = 0..299
    h    = ssp(e_k @ W1 + b1)                        [B,N,N,64]
    w_l  = ssp(h @ W2 + b2)                          [B,N,N,64]
    out  = sum_j x[b,i,:] * w_l[b,i,j,:]  = x[b,i,:] * sum_j g(d[b,i,j])
where g: scalar -> R^64 is a smooth analytic function of the distance alone
(ssp = softplus - log 2).

g is analytic on d in [0,1), so a degree-7 polynomial in u = 2d-1
approximates it to ~1e-3 (the host LS-fits the coefficient table against
the exact g on a dense grid, using the device's own bf16 tile functions as
the basis, so bf16 rounding bias is absorbed by the fit; final rel err
~5e-4).  The device evaluates the monomial product DAG in bf16 on the
Vector engine (bf16 tensor ops run in the DVE 2x perf mode), with the
free-dim j-reduction fused into every op via accum_out:
    u   = 2d - 1                      (tensor_scalar, fused sum -> P[:,b,1])
    t2  = u*u, t3 = u*t2, t4 = t2*t2, t5 = t2*t3, t6 = t3*t3, t7 = t3*t4
                                      (scalar_tensor_tensor, fused sums)
P[i, b, n] = sum_j u^n lands directly in SBUF with no reduction pass.
The tiny mixing S = P^T @ A runs per batch on the otherwise-idle PE
(transpose + 8x128 @ 8x64 matmul), then out = x * S.  The Scalar engine
issues half the DMAs (it fronts the second HWDGE ring) and runs no
ACTIVATE at all, so no activation-table load is paid.

Host-side, inputs are pre-transposed to the on-chip layout ([i, b, ...]) so
every DMA is a contiguous per-partition stream, and the output is
transposed back after the run.

Sharding: data-parallel over the batch dim B=16 -> 2 batches per core.
"""

import numpy as np

import concourse.bacc as bacc
import concourse.bass as bass
import concourse.masks as masks
import concourse.mybir as mybir
from concourse.bass_utils import run_bass_kernel_spmd
from concourse.tile import TileContext

F32 = mybir.dt.float32
BF16 = mybir.dt.bfloat16
ALU = mybir.AluOpType

N_CORES = 8
B, N, F = 16, 128, 64
B_LOC = B // N_CORES          # batches per core
N_RBF = 300
GAMMA = 10.0
LOG2 = float(np.log(2.0))

M_DEG = 7                     # polynomial degree of the fit
N_BASIS = M_DEG + 1           # constant + degrees 1..M

# product DAG: degree n -> (a, b) with n = a + b
_DAG = {n: (n // 2, n - n // 2) for n in range(2, M_DEG + 1)}


# ----------------------------------------------------------------------------
# Host-side: replicate the device bf16 tile DAG and LS-fit g in it
# ----------------------------------------------------------------------------

def _bf16(x):
    x = np.asarray(x, np.float32)
    u = x.view(np.uint32)
    r = ((u >> 16) & 1) + 0x7FFF          # round to nearest even
    return ((u + r) & 0xFFFF0000).view(np.float32)


def _dag_tiles(d, M):
    u = _bf16(2.0 * np.asarray(d, np.float32) - 1.0)
    t = {1: u}
    for n in range(2, M + 1):
        a, b = _DAG[n]
        t[n] = _bf16(t[a] * t[b])
    return t


def _coef_table(W1, b1, W2, b2):
    """A[n, f] so that g_f(d) ~= sum_n A[n, f] * tile_n(d) (float32)."""
    Q = 8192
    dq = np.linspace(0.0, 1.0, Q)

    centers = 0.1 * np.arange(N_RBF)
    e = np.exp(-GAMMA * (dq[:, None] - centers) ** 2)            # [Q, 300]

    def ssp(v):
        return np.logaddexp(0.0, v) - LOG2

    h = ssp(e @ W1.astype(np.float64) + b1.astype(np.float64))
    g = ssp(h @ W2.astype(np.float64) + b2.astype(np.float64))   # [Q, 64]

    tiles = _dag_tiles(dq, M_DEG)
    Bmat = np.stack([np.ones(Q)] +
                    [tiles[n].astype(np.float64)
                     for n in range(1, M_DEG + 1)], 1)           # [Q, N_BASIS]
    A, *_ = np.linalg.lstsq(Bmat, g, rcond=None)
    return np.ascontiguousarray(A, np.float32)


# ----------------------------------------------------------------------------
# Device kernel (per core), all I/O in on-chip layout:
#   d [N, B_LOC, N] (= [i, b, j]), x/y [N, B_LOC, F] (= [i, b, f])
# ----------------------------------------------------------------------------

_NC_CACHE = None


def _build_nc():
    nc = bacc.Bacc()

    d_in = nc.declare_dram_parameter("d", [N, B_LOC, N], F32, isOutput=False)
    x_in = nc.declare_dram_parameter("x", [N, B_LOC, F], F32, isOutput=False)
    a_in = nc.declare_dram_parameter("coef", [N_BASIS, F], F32, isOutput=False)
    y_out = nc.declare_dram_parameter("y", [N, B_LOC, F], F32, isOutput=True)

    with TileContext(nc) as tc:
        with (
            tc.sbuf_pool(name="sb", bufs=1) as sb,
            tc.psum_pool(name="ps", bufs=1) as ps,
        ):
            # ---- loads; d first (it gates all compute), one batch per
            # HWDGE ring (sync + scalar) so the two halves run in parallel
            d_sb = sb.tile([N, B_LOC, N], F32)        # [i, (b, j)]
            nc.sync.dma_start(out=d_sb[:, 0, :], in_=d_in[:, 0, :])
            nc.scalar.dma_start(out=d_sb[:, 1, :], in_=d_in[:, 1, :])
            x_sb = sb.tile([N, B_LOC, F], F32)        # [i, (b, f)]
            nc.sync.dma_start(out=x_sb[:, :, :], in_=x_in[:, :, :])
            a_sb = sb.tile([N_BASIS, F], F32)
            nc.scalar.dma_start(out=a_sb[:, :], in_=a_in[:, :])

            # identity for the P transposes, built on the idle GPSIMD
            id_sb = sb.tile([N, N], F32)
            masks.make_identity(nc, id_sb[:, :])

            # all-ones second operand for the u = 2d - 1 STT (tensor_scalar's
            # second scalar slot is repurposed by accum_out, so STT it is)
            ones_sb = sb.tile([N, B_LOC, N], F32)
            nc.gpsimd.memset(ones_sb[:, :, :], 1.0)

            # P[i, b, n] = sum_j tile_n[i, b, j]
            P_sb = sb.tile([N, B_LOC, N_BASIS], F32)
            nc.gpsimd.memset(P_sb[:, :, 0:1], float(N))   # constant basis

            # ---- bf16 monomial DAG with fused j-reduction (all DVE) ------
            t = {n: sb.tile([N, B_LOC, N], BF16, name=f"t{n}")
                 for n in range(1, M_DEG + 1)}
            for b in range(B_LOC):
                nc.vector.scalar_tensor_tensor(
                    t[1][:, b, :], d_sb[:, b, :], 2.0, ones_sb[:, b, :],
                    ALU.mult, ALU.subtract,
                    accum_out=P_sb[:, b, 1:2])
                for n in range(2, M_DEG + 1):
                    pa, pb = _DAG[n]
                    nc.vector.scalar_tensor_tensor(
                        t[n][:, b, :], t[pa][:, b, :], 1.0, t[pb][:, b, :],
                        ALU.mult, ALU.mult,
                        accum_out=P_sb[:, b, n:n + 1])

            # ---- per-batch tail: S = P^T @ A on PE, out = x * S ----------
            o_sb = sb.tile([N, B_LOC, F], F32)
            for b in range(B_LOC):
                pt_ps = ps.tile([N_BASIS, N], F32, space="PSUM",
                                name=f"pt_ps{b}")
                nc.tensor.transpose(pt_ps[:, :], P_sb[:, b, :], id_sb[:, :])
                pt_sb = sb.tile([N_BASIS, N], F32, name=f"pt_sb{b}")
                nc.vector.tensor_copy(pt_sb[:, :], pt_ps[:, :])
                s_ps = ps.tile([N, F], F32, space="PSUM", name=f"s_ps{b}")
                nc.tensor.matmul(s_ps[:, :], pt_sb[:, :], a_sb[:, :])
                nc.vector.tensor_tensor(o_sb[:, b, :], s_ps[:, :],
                                        x_sb[:, b, :], ALU.mult)
                # one output store per HWDGE ring
                eng = nc.sync if b == 0 else nc.scalar
                eng.dma_start(out=y_out[:, b, :], in_=o_sb[:, b, :])

    nc.compile()
    return nc


# ----------------------------------------------------------------------------
# Public entry point
# ----------------------------------------------------------------------------

def _run(x, distances, W1, b1, W2, b2, trace=False, **trace_kwargs):
    global _NC_CACHE
    x = np.asarray(x, np.float32)
    distances = np.asarray(distances, np.float32)

    A = _coef_table(W1, b1, W2, b2)                  # [N_BASIS, F]

    if _NC_CACHE is None:
        _NC_CACHE = _build_nc()
    nc = _NC_CACHE

    in_maps = []
    for c in range(N_CORES):
        sl = slice(c * B_LOC, (c + 1) * B_LOC)
        in_maps.append({
            # pre-transpose to the on-chip [i, b, ...] layout so the DMAs
            # stream contiguously into the partitions
            "d": np.ascontiguousarray(
                distances[sl].transpose(1, 0, 2)),   # [N, B_LOC, N]
            "x": np.ascontiguousarray(x[sl].transpose(1, 0, 2)),
            "coef": A,
        })

    res = run_bass_kernel_spmd(nc, in_maps, list(range(N_CORES)),
                               trace=trace, **trace_kwargs)
    y = np.concatenate(
        [res.results[c]["y"].transpose(1, 0, 2) for c in range(N_CORES)],
        axis=0)
    return np.ascontiguousarray(y), res


def kernel(x, distances, W1, b1, W2, b2):
    y, _ = _run(x, distances, W1, b1, W2, b2)
    return y


# revision 6
# speedup vs baseline: 1.2640x; 1.0090x over previous
"""CFConv (SchNet-style continuous-filter conv) kernel for 8 TRN2 NeuronCores.

Math: the reference computes
    e_k  = exp(-10*(d[b,i,j] - 0.1*k)^2)            k = 0..299
    h    = ssp(e_k @ W1 + b1)                        [B,N,N,64]
    w_l  = ssp(h @ W2 + b2)                          [B,N,N,64]
    out  = sum_j x[b,i,:] * w_l[b,i,j,:]  = x[b,i,:] * sum_j g(d[b,i,j])
where g: scalar -> R^64 is a smooth analytic function of the distance alone
(ssp = softplus - log 2).

g is analytic on d in [0,1), so a degree-7 polynomial in u = 2d-1
approximates it to ~1e-3 (the host LS-fits the coefficient table against
the exact g on a dense grid, using the device's own bf16 tile functions as
the basis, so bf16 rounding bias is absorbed by the fit).

Device data layout is transposed: d lives as [j, (b, i)], so the whole
j-reduction *and* the coefficient mixing collapse into 7 accumulating PE
matmuls over the (j, n) contraction:
    u   = 2d - 1                       (one 256-wide tensor_scalar, bf16 out)
    t2  = u*u, t3 = u*t2, t4 = t2*t2, t5 = t2*t3, t6 = t3*t3, t7 = t3*t4
                                       (256-wide bf16 STT, DVE 2x perf mode)
    S^T[f, (b,i)] += abc[:, n, :].T @ t_n     n = 1..7  (PE, bf16, PSUM acc)
    out = (S^T + c) * x^T              (one STT; c = N*A[0,:] rides as an
                                        extra fp32 column of the x upload)
abc[j, n, f] = A[n, f] replicated over j.  No on-chip transpose, no
reduction pass, no per-batch op splitting.  The Scalar engine fronts the
second HWDGE DMA ring and runs no ACTIVATE, so no activation-table load.

Host-side, inputs are pre-transposed to the on-chip layouts and the output
is transposed back after the run.

Sharding: data-parallel over the batch dim B=16 -> 2 batches per core.
"""

import numpy as np
import ml_dtypes

import concourse.bacc as bacc
import concourse.bass as bass
import concourse.mybir as mybir
from concourse.bass_utils import run_bass_kernel_spmd
from concourse.tile import TileContext

F32 = mybir.dt.float32
BF16 = mybir.dt.bfloat16
ALU = mybir.AluOpType

N_CORES = 8
B, N, F = 16, 128, 64
B_LOC = B // N_CORES          # batches per core
BI = B_LOC * N                # merged (b, i) free extent = 256
N_RBF = 300
GAMMA = 10.0
LOG2 = float(np.log(2.0))

M_DEG = 7                     # polynomial degree of the fit
N_BASIS = M_DEG + 1           # constant + degrees 1..M

# product DAG: degree n -> (a, b) with n = a + b
_DAG = {n: (n // 2, n - n // 2) for n in range(2, M_DEG + 1)}


# ----------------------------------------------------------------------------
# Host-side: replicate the device bf16 tile DAG and LS-fit g in it
# ----------------------------------------------------------------------------

def _bf16(x):
    x = np.asarray(x, np.float32)
    u = x.view(np.uint32)
    r = ((u >> 16) & 1) + 0x7FFF          # round to nearest even
    return ((u + r) & 0xFFFF0000).view(np.float32)


def _dag_tiles(d, M):
    u = _bf16(2.0 * np.asarray(d, np.float32) - 1.0)
    t = {1: u}
    for n in range(2, M + 1):
        a, b = _DAG[n]
        t[n] = _bf16(t[a] * t[b])
    return t


def _coef_table(W1, b1, W2, b2):
    """A[n, f] so that g_f(d) ~= sum_n A[n, f] * tile_n(d) (float64)."""
    Q = 8192
    dq = np.linspace(0.0, 1.0, Q)

    centers = 0.1 * np.arange(N_RBF)
    e = np.exp(-GAMMA * (dq[:, None] - centers) ** 2)            # [Q, 300]

    def ssp(v):
        return np.logaddexp(0.0, v) - LOG2

    h = ssp(e @ W1.astype(np.float64) + b1.astype(np.float64))
    g = ssp(h @ W2.astype(np.float64) + b2.astype(np.float64))   # [Q, 64]

    tiles = _dag_tiles(dq, M_DEG)
    Bmat = np.stack([np.ones(Q)] +
                    [tiles[n].astype(np.float64)
                     for n in range(1, M_DEG + 1)], 1)           # [Q, N_BASIS]
    A, *_ = np.linalg.lstsq(Bmat, g, rcond=None)
    return A                                                     # [N_BASIS, F]


# ----------------------------------------------------------------------------
# Device kernel (per core), all I/O in on-chip layout:
#   d [j, b, i], xc [f, (b i | c)], abc [j, n, f], y [f, b, i]
# ----------------------------------------------------------------------------

_NC_CACHE = None


def _build_nc():
    nc = bacc.Bacc()

    d_in = nc.declare_dram_parameter("d", [N, B_LOC, N], F32, isOutput=False)
    x_in = nc.declare_dram_parameter("xc", [F, BI + 1], F32, isOutput=False)
    a_in = nc.declare_dram_parameter("abc", [N, M_DEG, F], BF16,
                                     isOutput=False)
    y_out = nc.declare_dram_parameter("y", [F, B_LOC, N], F32, isOutput=True)

    with TileContext(nc) as tc:
        with (
            tc.sbuf_pool(name="sb", bufs=1) as sb,
            tc.psum_pool(name="ps", bufs=1) as ps,
        ):
            # ---- loads; d first (it gates all compute), one batch per
            # HWDGE ring (sync + scalar) so the two halves run in parallel
            d_sb = sb.tile([N, B_LOC, N], F32)        # [j, (b, i)]
            nc.sync.dma_start(out=d_sb[:, 0, :], in_=d_in[:, 0, :])
            nc.scalar.dma_start(out=d_sb[:, 1, :], in_=d_in[:, 1, :])
            # coefficient blocks, one half per ring
            a_sb = sb.tile([N, M_DEG, F], BF16)       # [j, n, f]
            half = M_DEG // 2
            nc.sync.dma_start(out=a_sb[:, :half, :], in_=a_in[:, :half, :])
            nc.scalar.dma_start(out=a_sb[:, half:, :], in_=a_in[:, half:, :])
            x_sb = sb.tile([F, BI + 1], F32)          # [f, (b i | c)]
            nc.scalar.dma_start(out=x_sb[:, :], in_=x_in[:, :])

            # ---- bf16 monomial DAG, merged 256-wide ops (all DVE),
            # with the (j, n)-contraction running on the PE as 7
            # accumulating matmuls into S^T [f, (b, i)]
            t = {n: sb.tile([N, BI], BF16, name=f"t{n}")
                 for n in range(1, M_DEG + 1)}
            s_ps = ps.tile([F, BI], F32, space="PSUM")

            nc.vector.tensor_scalar(
                t[1][:, :], d_sb.rearrange("j b i -> j (b i)"), 2.0, -1.0,
                ALU.mult, ALU.add)
            nc.tensor.matmul(s_ps[:, :], a_sb[:, 0, :], t[1][:, :],
                             start=True, stop=False)
            for n in range(2, M_DEG + 1):
                pa, pb = _DAG[n]
                nc.vector.scalar_tensor_tensor(
                    t[n][:, :], t[pa][:, :], 1.0, t[pb][:, :],
                    ALU.mult, ALU.mult)
                nc.tensor.matmul(s_ps[:, :], a_sb[:, n - 1, :], t[n][:, :],
                                 start=False, stop=(n == M_DEG))

            # ---- out = (S^T + c) * x^T in one STT, then store ------------
            o_sb = sb.tile([F, B_LOC, N], F32)
            nc.vector.scalar_tensor_tensor(
                o_sb.rearrange("f b i -> f (b i)"), s_ps[:, :],
                x_sb[:, BI:BI + 1], x_sb[:, 0:BI],
                ALU.add, ALU.mult)
            nc.sync.dma_start(out=y_out[:, 0, :], in_=o_sb[:, 0, :])
            nc.scalar.dma_start(out=y_out[:, 1, :], in_=o_sb[:, 1, :])

    nc.compile()
    return nc


# ----------------------------------------------------------------------------
# Public entry point
# ----------------------------------------------------------------------------

def _run(x, distances, W1, b1, W2, b2, trace=False, **trace_kwargs):
    global _NC_CACHE
    x = np.asarray(x, np.float32)
    distances = np.asarray(distances, np.float32)

    A = _coef_table(W1, b1, W2, b2)                  # [N_BASIS, F] float64
    abc = np.ascontiguousarray(
        np.broadcast_to(A[None, 1:, :], (N, M_DEG, F))
        .astype(ml_dtypes.bfloat16))                 # [j, n, f]
    c = (float(N) * A[0, :]).astype(np.float32)      # [F]

    if _NC_CACHE is None:
        _NC_CACHE = _build_nc()
    nc = _NC_CACHE

    in_maps = []
    for c_id in range(N_CORES):
        sl = slice(c_id * B_LOC, (c_id + 1) * B_LOC)
        xc = np.empty((F, BI + 1), np.float32)
        xc[:, :BI] = x[sl].transpose(2, 0, 1).reshape(F, BI)
        xc[:, BI] = c
        in_maps.append({
            # pre-transpose to the on-chip layouts so the DMAs stream
            # contiguously into the partitions
            "d": np.ascontiguousarray(
                distances[sl].transpose(2, 0, 1)),   # [j, b, i]
            "xc": xc,
            "abc": abc,
        })

    res = run_bass_kernel_spmd(nc, in_maps, list(range(N_CORES)),
                               trace=trace, **trace_kwargs)
    y = np.concatenate(
        [res.results[c_id]["y"].transpose(1, 2, 0) for c_id in range(N_CORES)],
        axis=0)
    return np.ascontiguousarray(y), res


def kernel(x, distances, W1, b1, W2, b2):
    y, _ = _run(x, distances, W1, b1, W2, b2)
    return y
